# revision 19
# baseline (speedup 1.0000x reference)
"""Trainium2 kernel for nn_LJCH1_34548716929306 (ragged_sequence).

Strategy (pure data-parallel over batch, per sharding hint):
  - The dominant cost is the fc0 projection: concat([featContent,
    featDistort, motionFeat]) [16,2048,4864] @ fc0_w.T [4864,128].
    It runs on the 8 NeuronCores, 2 samples per core, as
    scores.T = wT.T @ xT with feature-major (K-major) layout prepared
    host-side so the device streams contiguous tiles with zero on-chip
    transposes.
  - Device kernel: K-outer accumulation into a single [128, 4096] fp32
    PSUM block spanning all 8 banks; weights loaded once as a single
    [128, 4864] stationary-source SBUF tile; x streamed in 19 chunks of
    [128, 2, 4096] (256 contraction rows each) with 4-deep
    double-buffering; one PSUM->SBUF copy + one output DMA at the end.
  - Precision: weights stay bf16 (stationary); the streamed features
    use fp8 e3m4 (X_DT="f8e3"), halving HBM traffic vs bf16. Measured
    end-to-end rel-err ~1.1e-2 vs the fp32 reference (gate 2e-2);
    X_DT="bf16" is the conservative fallback (~1.5e-3).
  - The BiGRU over T=2048 (H=32) and the masked multi-scale softmax
    head are tiny (~0.1% of FLOPs) and sequential; they run in fp32
    numpy on host.
"""

import numpy as np
import ml_dtypes
from concurrent.futures import ThreadPoolExecutor

import concourse.bass as bass
import concourse.bacc as bacc
import concourse.tile as tile
from concourse import mybir
from concourse.bass_utils import run_bass_kernel_spmd

B, T = 16, 2048
D_CONTENT, D_DISTORT, D_MOTION = 4096, 512, 256
D = D_CONTENT + D_DISTORT + D_MOTION  # 4864
RED, H = 128, 32
N_CORES = 8
BL = B // N_CORES  # 2 samples per core
N = BL * T  # 4096 columns per core
KP = 128  # contraction rows per matmul
KB = D // KP  # 38 k-blocks
CH = KB // 2  # 19 DMA chunks of 2 k-blocks (256 rows, 1-2MB each)
NB = N // 512  # 8 psum banks
TIME_INTERVAL = 2
NEG = -1e9

# Streamed-feature dtype: "f8e3" (fp8 e3m4, fast) or "bf16" (conservative).
X_DT = "f8e3"


def _x_np_dt():
    return ml_dtypes.float8_e3m4 if X_DT == "f8e3" else ml_dtypes.bfloat16


def _x_bir_dt():
    return mybir.dt.float8e3 if X_DT == "f8e3" else mybir.dt.bfloat16


_compiled = None


# Weight DMA split: kb-ranges per chunk so the first matmul only waits on a
# ~330KB weight chunk + the first x chunk instead of the full 1.25MB.
W_CHUNKS = (10, 10, 10, 8)
# 1024-wide moving operands are rejected by walrus codegen
# (s3d3_mm_num_elements): matmul free dim is capped at 512 (one PSUM bank).
MOVING_1024 = False
# Dummy matmuls issued during the DMA fill to ramp the PE HAM clock gate.
N_WARMUP_MM = 8


def _build_nc():
    nc = bacc.Bacc(
        "TRN2",
        target_bir_lowering=False,
        debug=False,
        enable_asserts=False,
        num_devices=1,  # no partition-id machinery; pure SPMD across cores
    )
    xdt = _x_bir_dt()
    wdt = mybir.dt.bfloat16
    f32 = mybir.dt.float32
    xT = nc.dram_tensor("xT", [CH, KP, 2, N], xdt, kind="ExternalInput")
    wT = nc.dram_tensor("wT", [KP, D], wdt, kind="ExternalInput")
    sT = nc.dram_tensor("sT", [RED, N], mybir.dt.bfloat16, kind="ExternalOutput")
    with tile.TileContext(nc) as tc:
        with tc.tile_pool(name="wp", bufs=len(W_CHUNKS)) as wp, \
             tc.tile_pool(name="xp", bufs=6) as xp, \
             tc.tile_pool(name="dp", bufs=1) as dp, \
             tc.tile_pool(name="pp", bufs=1, space="PSUM") as pp, \
             tc.tile_pool(name="op", bufs=NB) as op:
            # First weight chunk + first x chunk lead the DMA queue.
            w_tiles, w_start = [], []
            kb0 = 0
            for wc, nkb in enumerate(W_CHUNKS):
                w_tiles.append(wp.tile([KP, nkb * KP], wdt, name=f"wchunk{wc}", tag="wchunk"))
                w_start.append(kb0)
                kb0 += nkb

            def lhsT(kb):
                for wc in range(len(W_CHUNKS) - 1, -1, -1):
                    if kb >= w_start[wc]:
                        off = (kb - w_start[wc]) * KP
                        return w_tiles[wc][:, off:off + KP]
                raise AssertionError

            # DMA order: w0, x0 (split per-j so the first matmuls wait on
            # 512KB, not 1MB), x1, x2, then the remaining weight chunks
            # (first needed ~25% into the stream), then x3...
            nc.sync.dma_start(w_tiles[0], wT.ap()[:, :W_CHUNKS[0] * KP])
            x0a = xp.tile([KP, N], xdt, name="xchunk0a", tag="xchunk")
            nc.sync.dma_start(x0a, xT.ap()[0][:, 0])
            x0b = xp.tile([KP, N], xdt, name="xchunk0b", tag="xchunk")
            nc.sync.dma_start(x0b, xT.ap()[0][:, 1])
            # Later weight chunks interleave sparsely into the x stream so
            # they don't starve the first kb's of moving data (each lands
            # well before its kb range is consumed).
            x_tiles = {}
            w_after = {3: 1, 4: 2, 5: 3}  # x-chunk index -> w-chunk to issue after

            def issue_x(c):
                x_tiles[c] = xp.tile([KP, 2, N], xdt, name=f"xchunk{c}", tag="xchunk")
                nc.sync.dma_start(x_tiles[c], xT.ap()[c])
                wc = w_after.get(c)
                if wc is not None:
                    col = sum(W_CHUNKS[:wc]) * KP
                    nc.sync.dma_start(w_tiles[wc], wT.ap()[:, col:col + W_CHUNKS[wc] * KP])

            for c in (1, 2, 3, 4):
                issue_x(c)

            MV = 1024 if MOVING_1024 else 512
            NMV = N // MV
            ps = pp.tile([RED, N], f32)

            # HAM warmup: the PE clock sits at 1.2GHz until ~3.4us of
            # sustained activity. Dummy matmuls (zeroed operands, results
            # discarded by the real start=True group) run during the DMA
            # fill so the real stream starts at 2.4GHz.
            dummy = dp.tile([KP, 512], xdt)
            nc.gpsimd.memset(dummy, 0)
            for _ in range(N_WARMUP_MM):
                nc.tensor.matmul(
                    ps[:, :512], lhsT=dummy[:, :KP], rhs=dummy,
                    start=True, stop=True,
                )
            for c in range(CH):
                if c == 0:
                    xjs = (x0a, x0b)
                else:
                    if c not in x_tiles:
                        issue_x(c)
                    xt = x_tiles.pop(c)
                    xjs = (xt[:, 0], xt[:, 1])
                if c < CH - 1:
                    # kb-major: both j's of this chunk across all banks
                    for j in range(2):
                        kb = 2 * c + j
                        for n in range(NMV):
                            nc.tensor.matmul(
                                ps[:, n * MV:(n + 1) * MV],
                                lhsT=lhsT(kb),
                                rhs=xjs[j][:, n * MV:(n + 1) * MV],
                                start=(kb == 0),
                                stop=False,
                            )
                else:
                    # Last chunk bank-major: bank n finishes, its cast + store
                    # overlap the remaining banks' matmuls.
                    for n in range(NMV):
                        for j in range(2):
                            kb = 2 * c + j
                            nc.tensor.matmul(
                                ps[:, n * MV:(n + 1) * MV],
                                lhsT=lhsT(kb),
                                rhs=xjs[j][:, n * MV:(n + 1) * MV],
                                start=False,
                                stop=(j == 1),
                            )
                        ot = op.tile([RED, MV], mybir.dt.bfloat16, name=f"obank{n}", tag="obank")
                        nc.vector.tensor_copy(ot, ps[:, n * MV:(n + 1) * MV])
                        nc.sync.dma_start(sT.ap()[:, n * MV:(n + 1) * MV], ot)
    nc.compile()
    return nc


def _get_compiled():
    global _compiled
    if _compiled is None:
        _compiled = _build_nc()
    return _compiled


_runner = None


def _get_runner():
    """Build the sharded PJRT executable once and reuse it across calls.

    run_bass_kernel_spmd's axon path re-traces and re-jits the shard_map
    wrapper on every invocation (fresh closures defeat the jit cache);
    caching it here removes multi-second per-call overhead.
    """
    global _runner
    if _runner is not None:
        return _runner
    import jax
    from jax.sharding import Mesh, PartitionSpec
    from jax.experimental.shard_map import shard_map
    from concourse import bass2jax
    from concourse import mybir as _mybir

    nc = _get_compiled()
    bass2jax.install_neuronx_cc_hook()

    partition_name = nc.partition_id_tensor.name if nc.partition_id_tensor else None
    in_names, out_names, out_avals = [], [], []
    for alloc in nc.m.functions[0].allocations:
        if not isinstance(alloc, _mybir.MemoryLocationSet):
            continue
        name = alloc.memorylocations[0].name
        if alloc.kind == "ExternalInput":
            if name != partition_name:
                in_names.append(name)
        elif alloc.kind == "ExternalOutput":
            out_names.append(name)
            out_avals.append(
                jax.core.ShapedArray(tuple(alloc.tensor_shape), _mybir.dt.np(alloc.dtype))
            )
    n_params = len(in_names)
    n_outs = len(out_avals)
    all_in_names = list(in_names) + list(out_names)
    if partition_name is not None:
        all_in_names.append(partition_name)
    donate = tuple(range(n_params, n_params + n_outs))

    def _body(*args):
        operands = list(args)
        if partition_name is not None:
            operands.append(bass2jax.partition_id_tensor())
        outs = bass2jax._bass_exec_p.bind(
            *operands,
            out_avals=tuple(out_avals),
            in_names=tuple(all_in_names),
            out_names=tuple(out_names),
            lowering_input_output_aliases=(),
            sim_require_finite=True,
            sim_require_nnan=True,
            nc=nc,
        )
        return tuple(outs)

    devices = jax.devices()[:N_CORES]
    mesh = Mesh(np.asarray(devices), ("core",))
    in_specs = (PartitionSpec("core"),) * (n_params + n_outs)
    out_specs = (PartitionSpec("core"),) * n_outs
    sharded = jax.jit(
        shard_map(_body, mesh=mesh, in_specs=in_specs, out_specs=out_specs,
                  check_rep=False),
        donate_argnums=donate,
        keep_unused=True,
    )

    def run(in_maps):
        concat_in = [
            np.concatenate([np.asarray(m[name]) for m in in_maps], axis=0)
            for name in in_names
        ]
        concat_zeros = [
            np.zeros((N_CORES * a.shape[0], *a.shape[1:]), a.dtype) for a in out_avals
        ]
        out_arrs = sharded(*concat_in, *concat_zeros)
        return [
            {
                name: np.asarray(out_arrs[i]).reshape(N_CORES, *out_avals[i].shape)[c]
                for i, name in enumerate(out_names)
            }
            for c in range(N_CORES)
        ]

    _runner = run
    # expose pieces for external timing/inspection (test harness use)
    global _sharded, _mesh, _in_names_g, _out_names_g, _out_avals_g
    _sharded, _mesh = sharded, mesh
    _in_names_g, _out_names_g, _out_avals_g = in_names, out_names, out_avals
    return _runner


def _run_device(in_maps):
    try:
        return _get_runner()(in_maps)
    except Exception:
        return run_bass_kernel_spmd(_get_compiled(), in_maps, list(range(N_CORES))).results


def _sigmoid(x):
    return 1.0 / (1.0 + np.exp(-x))


def _gru_dir(gi, wh, bh, reverse):
    # gi: [T, B, 3H] precomputed input gates; returns ys [T, B, H]
    Tn, Bn, _ = gi.shape
    whT = wh.T.copy()  # [H, 3H]
    h = np.zeros((Bn, H), np.float32)
    ys = np.empty((Tn, Bn, H), np.float32)
    order = range(Tn - 1, -1, -1) if reverse else range(Tn)
    for t in order:
        g = gi[t]
        gh = h @ whT + bh
        i_r, i_z, i_n = g[:, :H], g[:, H : 2 * H], g[:, 2 * H :]
        h_r, h_z, h_n = gh[:, :H], gh[:, H : 2 * H], gh[:, 2 * H :]
        r = _sigmoid(i_r + h_r)
        z = _sigmoid(i_z + h_z)
        n = np.tanh(i_n + r * h_n)
        h = (1.0 - z) * n + z * h
        ys[t] = h
    return ys


def _conv1d_same(x, w):
    # cross-correlation with zero 'same' padding; x [B,T], w [k]
    k = w.shape[0]
    p = k // 2
    xp = np.pad(x, ((0, 0), (p, p)))
    out = np.zeros_like(x)
    for j in range(k):
        out += w[j] * xp[:, j : j + x.shape[1]]
    return out


def _make_in_maps(inputs):
    fC = np.asarray(inputs["featContent"], np.float32)
    fD = np.asarray(inputs["featDistort"], np.float32)
    mF = np.asarray(inputs["motionFeat"], np.float32)
    fc0_w = np.asarray(inputs["fc0_w"], np.float32)
    # Weights: partition-major packed [128, 4864] bf16 so the device loads
    # them with a single line-rate DMA. wPack[p, kb*128+m] = fc0_w[m, kb*128+p].
    wPack = np.ascontiguousarray(
        fc0_w.T.reshape(KB, KP, RED).transpose(1, 0, 2).reshape(KP, D)
    ).astype(ml_dtypes.bfloat16)
    xdt = _x_np_dt()

    def build(c):
        sl = slice(c * BL, (c + 1) * BL)
        # Feature-major x for this core's BL samples: [D, N]
        xTf = np.empty((D, N), np.float32)
        xTf[:D_CONTENT] = fC[sl].reshape(N, D_CONTENT).T
        xTf[D_CONTENT:D_CONTENT + D_DISTORT] = fD[sl].reshape(N, D_DISTORT).T
        xTf[D_CONTENT + D_DISTORT:] = mF[sl].reshape(N, D_MOTION).T
        if X_DT == "f8e3":
            np.clip(xTf, -15.5, 15.5, out=xTf)
        # chunked layout [CH, 128, 2, N]: k = c2*256 + j*128 + p
        xBig = np.ascontiguousarray(
            xTf.reshape(CH, 2, KP, N).transpose(0, 2, 1, 3)
        ).astype(xdt)
        return {"xT": xBig, "wT": wPack}

    with ThreadPoolExecutor(N_CORES) as ex:
        return list(ex.map(build, range(N_CORES)))


def kernel(**inputs):
    inputLength = np.asarray(inputs["inputLength"])
    fc0_b = np.asarray(inputs["fc0_b"], np.float32)

    in_maps = _make_in_maps(inputs)
    results = _run_device(in_maps)

    scores = np.empty((B, T, RED), np.float32)
    for c in range(N_CORES):
        sT = results[c]["sT"]  # [RED, N] bf16
        scores[c * BL : (c + 1) * BL] = (
            sT.astype(np.float32).T.reshape(BL, T, RED)
        )
    scores += fc0_b

    # BiGRU (fp32 host)
    x_tbd = scores.transpose(1, 0, 2)  # [T,B,RED]
    gi_f = x_tbd @ np.asarray(inputs["gru_wi_f"], np.float32).T + np.asarray(
        inputs["gru_bi_f"], np.float32
    )
    gi_b = x_tbd @ np.asarray(inputs["gru_wi_b"], np.float32).T + np.asarray(
        inputs["gru_bi_b"], np.float32
    )
    yf = _gru_dir(gi_f, np.asarray(inputs["gru_wh_f"], np.float32),
                  np.asarray(inputs["gru_bh_f"], np.float32), reverse=False)
    yb = _gru_dir(gi_b, np.asarray(inputs["gru_wh_b"], np.float32),
                  np.asarray(inputs["gru_bh_b"], np.float32), reverse=True)
    outputs = np.concatenate([yf, yb], -1).transpose(1, 0, 2)  # [B,T,2H]

    q_w = np.asarray(inputs["q_w"], np.float32)
    q_b = np.asarray(inputs["q_b"], np.float32)
    q = (outputs @ q_w.T + q_b)[..., 0]  # [B,T]

    lengths = inputLength.astype(np.int64) - 2 * (TIME_INTERVAL // 2) - 1
    mask = np.arange(T)[None, :] < lengths[:, None]
    qm = np.where(mask, q, 0.0).astype(np.float32)

    total = np.zeros((B,), np.float32)
    for wk in ("w1", "w2", "w3"):
        w = np.asarray(inputs[wk], np.float32)
        logits = np.where(mask, _conv1d_same(qm, w), NEG).astype(np.float32)
        m = logits.max(-1, keepdims=True)
        e = np.exp(logits - m)
        sm = e / e.sum(-1, keepdims=True)
        total = total + (sm * qm).sum(-1)
    return (total / 3.0)[:, None].astype(np.float32)
